# revision 16
# baseline (speedup 1.0000x reference)
# Multi-head attention (B=4, S=2048, D=1024, H=16) on 8 TRN2 NeuronCores.
#
# Sharding: core c handles batch b = c//2 and head-group g = c%2 (8 heads).
# Each core computes Q/K/V projections for its 8 heads, full attention over
# S=2048, and a partial output projection over its 512 value-features.
# Host sums the two partial outputs per batch and adds b_out.
#
# Device math (per core, all matmuls bf16 with fp32 PSUM accumulation):
#   Q^T, K^T  [512, S]   = W_slice @ x^T (+ per-partition bias)
#   V         [S, 512]   = x @ Wv^T  (bias folded into values, see below)
#   S^T tiles [128k, q]  = K_h Q_h^T, two heads row-tiled on the PE array
#   P^T       = exp(S^T * 0.125)                 (ACT engine, bf16 out)
#   [PV^T; l] = [V | 1]^T @ P^T                  (l = softmax denominator)
#   values^T  = PV^T * (1/l) + b_v               (exact: (P(V+b))/l = PV/l + b)
#   out^T     = W_out_slice^T-contraction over 512 features (partial)
#
# v2: software-pipelined emission. The PE queue is strict-FIFO per engine, so
# a semaphore-waiting matmul blocks everything behind it. Emission order per
# kt step is: ss(kt+1) [only waits exp(kt-1)], filler unit, pv(kt) [waits
# exp(kt)]. Projection/out-proj work is split into ~0.4-0.9us units placed
# one per kt so no single filler blob stalls the exp pipeline. reciprocal ->
# reciprocal_approx_fast (the exact one is ~3.3us on one partition and
# clogged the DVE queue, delaying the PSUM-releasing copies behind it).
import sys

for _p in ("/opt/trn_rl_repo",):
    if _p not in sys.path:
        sys.path.insert(0, _p)

import numpy as np
import ml_dtypes

BF16 = ml_dtypes.bfloat16

B, S, D = 4, 2048, 1024
H, HD = 16, 64
P = 128
HC = 8            # heads per core
DC = HC * HD      # 512 value-features per core
KS = D // P       # 8 contraction subtiles for projections
MT = DC // P      # 4 M-tiles == head pairs
ST = S // P       # 16 seq tiles
NQ = 512          # q-chunk (matmul free dim)
QC = S // NQ      # 4
OT = D // P       # 8 output e-tiles
OKS = DC // P     # 4 contraction subtiles for out-proj


def build_nc(s=S):
    """Build the single-core Bass/Tile program (SPMD across 8 cores)."""
    import concourse.tile as tile
    import concourse.mybir as mybir
    from concourse import bacc
    from contextlib import ExitStack

    dt = mybir.dt
    st_n = s // P
    qc_n = s // NQ

    nc = bacc.Bacc("TRN2", target_bir_lowering=False, debug=False)

    xT = nc.dram_tensor("xT", [P, KS, s], dt.bfloat16, kind="ExternalInput").ap()
    wq = nc.dram_tensor("wq", [P, KS, DC], dt.bfloat16, kind="ExternalInput").ap()
    wk = nc.dram_tensor("wk", [P, KS, DC], dt.bfloat16, kind="ExternalInput").ap()
    wv = nc.dram_tensor("wv", [P, KS, DC], dt.bfloat16, kind="ExternalInput").ap()
    wo = nc.dram_tensor("wo", [P, OKS, D], dt.bfloat16, kind="ExternalInput").ap()
    bq = nc.dram_tensor("bq", [P, MT], dt.float32, kind="ExternalInput").ap()
    bk = nc.dram_tensor("bk", [P, MT], dt.float32, kind="ExternalInput").ap()
    bv = nc.dram_tensor("bv", [P, MT], dt.float32, kind="ExternalInput").ap()
    out = nc.dram_tensor("out", [P, OT, s], dt.float32, kind="ExternalOutput").ap()

    with tile.TileContext(nc) as tc, ExitStack() as ctx:
        persist = ctx.enter_context(tc.tile_pool(name="persist", bufs=1))
        # PSUM budget is 8 banks: scores 2x[128,1024] (4) + one shared 4-slot
        # pool of 1-bank tiles for proj/pv/outproj accumulators (4).
        ps_s = ctx.enter_context(tc.tile_pool(name="ps_s", bufs=2, space="PSUM"))
        ps_pv = ctx.enter_context(tc.tile_pool(name="ps_pv", bufs=4, space="PSUM"))
        ps_io = ps_pv
        ptp = ctx.enter_context(tc.tile_pool(name="ptp", bufs=6))
        small = ctx.enter_context(tc.tile_pool(name="small", bufs=6))
        dramp = ctx.enter_context(tc.tile_pool(name="dramp", bufs=4, space="DRAM"))

        xT_sb = persist.tile([P, KS, s], dt.bfloat16, name="xT_sb")
        wq_sb = persist.tile([P, KS, DC], dt.bfloat16, name="wq_sb")
        wk_sb = persist.tile([P, KS, DC], dt.bfloat16, name="wk_sb")
        wv_sb = persist.tile([P, KS, DC], dt.bfloat16, name="wv_sb")
        wo_sb = persist.tile([P, OKS, D], dt.bfloat16, name="wo_sb")
        bq_sb = persist.tile([P, MT], dt.float32, name="bq_sb")
        bk_sb = persist.tile([P, MT], dt.float32, name="bk_sb")
        bv_sb = persist.tile([P, MT], dt.float32, name="bv_sb")
        QT_sb = persist.tile([P, MT, s], dt.bfloat16, name="QT_sb")
        KT_sb = persist.tile([P, MT, s], dt.bfloat16, name="KT_sb")
        V_sb = persist.tile([P, st_n, HC, HD + 1], dt.bfloat16, name="V_sb")
        VL_sb = persist.tile([P, OKS, s], dt.bfloat16, name="VL_sb")

        # Input loads spread across the three DMA-capable queues (sync,
        # scalar, gpsimd) in dependency order, so the first K-projection can
        # start after ~5us instead of waiting out one serialized queue.
        half = s // 2
        for ks in range(KS):
            nc.sync.dma_start(out=wk_sb[:, ks], in_=wk[:, ks])
        nc.sync.dma_start(out=bk_sb, in_=bk)
        nc.sync.dma_start(out=bq_sb, in_=bq)
        for ks in range(KS):
            nc.scalar.dma_start(out=xT_sb[:, ks, 0:half], in_=xT[:, ks, 0:half])
            nc.gpsimd.dma_start(out=xT_sb[:, ks, half:s], in_=xT[:, ks, half:s])
        nc.scalar.dma_start(out=wq_sb, in_=wq)
        nc.gpsimd.dma_start(out=wv_sb, in_=wv)
        nc.gpsimd.dma_start(out=bv_sb, in_=bv)
        nc.scalar.dma_start(out=wo_sb, in_=wo)
        # ones column for the fused softmax-denominator trick
        nc.vector.memset(V_sb[:, :, :, HD:HD + 1], 1.0)

        # ---------- fine-grained filler units (~0.4-0.9us of PE work) ------
        proj_ps = {}

        def proj_qk_half(w_sb, b_sb, dst, mt, c, half, wch):
            # half 0: ks 0-3 (opens the psum group); half 1: ks 4-7 + bias.
            key = (wch, mt, c)
            if half == 0:
                proj_ps[key] = ps_io.tile(
                    [P, NQ], dt.float32, name=f"ps_{wch}_{mt}_{c}", tag="pv"
                )
            ps = proj_ps[key]
            for ks in range(4 * half, 4 * half + 4):
                nc.tensor.matmul(
                    ps,
                    lhsT=w_sb[:, ks, mt * P:(mt + 1) * P],
                    rhs=xT_sb[:, ks, c * NQ:(c + 1) * NQ],
                    start=(ks == 0),
                    stop=(ks == KS - 1),
                    skip_group_check=True,
                )
            if half == 1:
                nc.vector.tensor_add(
                    dst[:, mt, c * NQ:(c + 1) * NQ],
                    ps,
                    b_sb[:, mt:mt + 1].to_broadcast((P, NQ)),
                )
                del proj_ps[key]

        def proj_v(st):
            # V[s-tile, :] = x @ Wv^T (no bias here; folded into values)
            ps = ps_io.tile([P, DC], dt.float32, name=f"ps_v_{st}", tag="pv")
            for ks in range(KS):
                nc.tensor.matmul(
                    ps,
                    lhsT=xT_sb[:, ks, st * P:(st + 1) * P],
                    rhs=wv_sb[:, ks, :],
                    start=(ks == 0),
                    stop=(ks == KS - 1),
                )
            nc.vector.tensor_copy(
                out=V_sb[:, st, :, 0:HD],
                in_=ps.rearrange("p (h d) -> p h d", h=HC),
            )

        def outproj_et(c, et):
            # partial out-projection over this core's 512 value-features
            po = ps_io.tile([P, NQ], dt.float32, name=f"po_{et}_{c}", tag="pv")
            for ks in range(OKS):
                nc.tensor.matmul(
                    po,
                    lhsT=wo_sb[:, ks, et * P:(et + 1) * P],
                    rhs=VL_sb[:, ks, c * NQ:(c + 1) * NQ],
                    start=(ks == 0),
                    stop=(ks == OKS - 1),
                )
            ot_sb = small.tile([P, NQ], dt.float32, name=f"ot_{et}_{c}", tag="ot")
            nc.vector.tensor_copy(out=ot_sb, in_=po)
            eng = nc.sync if et % 2 == 0 else nc.gpsimd
            eng.dma_start(out=out[:, et, c * NQ:(c + 1) * NQ], in_=ot_sb)

        # ------------------- attention pipeline pieces ---------------------
        positions = [(pr, c, kt)
                     for pr in range(MT) for c in range(qc_n)
                     for kt in range(st_n)]
        npos = len(positions)
        ss_tiles = {}
        pt_tiles = {}
        pv_tiles = {}

        def emit_ss(i):
            pr, c, kt = positions[i]
            cs = slice(c * NQ, (c + 1) * NQ)
            ks_sl = slice(kt * P, (kt + 1) * P)
            ss = ps_s.tile([P, 2 * NQ], dt.float32, name=f"ss_{i}", tag="ss")
            ss_tiles[i] = ss
            # S^T = K_h Q_h^T for the two heads, row-tiled (K=64 each)
            nc.tensor.matmul(
                ss[:, 0:NQ],
                lhsT=KT_sb[0:HD, pr, ks_sl],
                rhs=QT_sb[0:HD, pr, cs],
            )
            nc.tensor.matmul(
                ss[:, NQ:2 * NQ],
                lhsT=KT_sb[HD:P, pr, ks_sl],
                rhs=QT_sb[HD:P, pr, cs],
            )

        def emit_exp(i):
            pt = ptp.tile([P, 2 * NQ], dt.bfloat16, name=f"pt_{i}", tag="pt")
            pt_tiles[i] = pt
            nc.scalar.activation(
                pt, ss_tiles.pop(i), mybir.ActivationFunctionType.Exp, scale=0.125
            )

        def emit_pv(i):
            pr, c, kt = positions[i]
            if kt == 0:
                pv_tiles[(pr, c)] = (
                    ps_pv.tile([HD + 1, NQ], dt.float32, name=f"pvA_{pr}_{c}", tag="pv"),
                    ps_pv.tile([HD + 1, NQ], dt.float32, name=f"pvB_{pr}_{c}", tag="pv"),
                )
            pvA, pvB = pv_tiles[(pr, c)]
            pt = pt_tiles.pop(i)
            nc.tensor.matmul(
                pvA,
                lhsT=V_sb[:, kt, 2 * pr, :],
                rhs=pt[:, 0:NQ],
                start=(kt == 0),
                stop=(kt == st_n - 1),
            )
            nc.tensor.matmul(
                pvB,
                lhsT=V_sb[:, kt, 2 * pr + 1, :],
                rhs=pt[:, NQ:2 * NQ],
                start=(kt == 0),
                stop=(kt == st_n - 1),
            )

        def normalize(pr, c, last=False):
            # values^T = PV^T * (1/l) + b_v; head B is DMA-shifted to
            # partitions 64..127 so out-proj sees [128, s] rhs tiles.
            # Both PSUM banks are evacuated FIRST (frees them for the next
            # chunk's accumulators), then the two slow reciprocals, then the
            # DMA-broadcast bounce; the SBUF-only muls/adds run on the idle
            # GPSIMD engine so they can't clog the DVE queue behind them.
            cs = slice(c * NQ, (c + 1) * NQ)
            pvA, pvB = pv_tiles.pop((pr, c))
            pvsA = small.tile([HD + 1, NQ], dt.float32, name=f"pvs_{pr}_{c}_0", tag="pvs")
            pvsB = small.tile([HD + 1, NQ], dt.float32, name=f"pvs_{pr}_{c}_1", tag="pvs")
            nc.vector.tensor_copy(out=pvsA, in_=pvA)
            nc.vector.tensor_copy(out=pvsB, in_=pvB)
            # Both l rows live on partition 64 of their psum tiles; DMA-gather
            # them onto two distinct partitions so ONE reciprocal handles both
            # (the iterative divide costs ~3.4us per 512 elems per lane no
            # matter how many lanes run, so batching halves the DVE cost).
            lr = small.tile([2, NQ], dt.float32, name=f"lr_{pr}_{c}", tag="lr")
            nc.sync.dma_start(out=lr[0:1, :], in_=pvsA[HD:HD + 1, :])
            nc.sync.dma_start(out=lr[1:2, :], in_=pvsB[HD:HD + 1, :])
            rec = small.tile([2, NQ], dt.float32, name=f"r_{pr}_{c}", tag="rec")
            nc.vector.reciprocal(rec, lr)
            ld = dramp.tile([2, NQ], dt.float32, name=f"ld_{pr}_{c}", tag="ld")
            nc.sync.dma_start(out=ld, in_=rec)
            for half, pvs in ((0, pvsA), (1, pvsB)):
                # DMA-broadcast 1/l down to partitions 0..63 (SBUF APs can't
                # have stride-0 partition dims, DRAM APs can: bounce via DRAM).
                bc = small.tile([HD, NQ], dt.float32,
                                name=f"bc_{pr}_{c}_{half}", tag="bc")
                nc.sync.dma_start(out=bc, in_=ld[half:half + 1, :].to_broadcast((HD, NQ)))
                ee = nc.vector if last else nc.gpsimd
                if half == 0:
                    ee.tensor_mul(VL_sb[0:HD, pr, cs], pvs[0:HD, :], bc)
                else:
                    stg = small.tile([HD, NQ], dt.bfloat16,
                                     name=f"st_{pr}_{c}", tag="stg")
                    ee.tensor_mul(stg, pvs[0:HD, :], bc)
                    nc.sync.dma_start(out=VL_sb[HD:P, pr, cs], in_=stg)
            (nc.vector if last else nc.gpsimd).tensor_add(
                VL_sb[:, pr, cs],
                VL_sb[:, pr, cs],
                bv_sb[:, pr:pr + 1].to_broadcast((P, NQ)),
            )

        # ------------------- filler worklists ------------------------------
        # One unit popped per kt. Deadlines honored by construction:
        #  - K(pr+1), Q(pr+1, c0) finish during pair pr
        #  - Q(pr, c+1) finishes during chunk (pr, c)
        #  - V is emitted just-in-time inside pair 0 chunk 0 (not a unit)
        #  - out-proj of query-chunk c-1 runs through pair 3's chunk c
        def qk_units(pr, c):
            us = []
            if c + 1 < qc_n:  # this pair's next q-chunk
                us += [lambda h=h, cc=c + 1: proj_qk_half(
                    wq_sb, bq_sb, QT_sb, pr, cc, h, "q") for h in range(2)]
            if pr + 1 < MT:
                if c < 2:     # next pair's K, spread over chunks 0-1
                    for kc in (2 * c, 2 * c + 1):
                        us += [lambda h=h, kc=kc: proj_qk_half(
                            wk_sb, bk_sb, KT_sb, pr + 1, kc, h, "k")
                            for h in range(2)]
                elif c == 2:  # next pair's first q-chunk
                    us += [lambda h=h: proj_qk_half(
                        wq_sb, bq_sb, QT_sb, pr + 1, 0, h, "q")
                        for h in range(2)]
            if pr == MT - 1 and c >= 1:
                # defer to the chunk's second half: the previous chunk's
                # normalize chain (evac -> recip -> bounce -> mul) needs ~8us
                # before VL is readable; an early out-proj would sem-block
                # the PE queue on it
                us += [None] * (8 - len(us))
                us += [lambda et=et, cc=c - 1: outproj_et(cc, et)
                       for et in range(OT)]
            return us

        # ------------------- preamble --------------------------------------
        for c in range(qc_n):
            for h in range(2):
                proj_qk_half(wk_sb, bk_sb, KT_sb, 0, c, h, "k")
        for h in range(2):
            proj_qk_half(wq_sb, bq_sb, QT_sb, 0, 0, h, "q")

        # ------------------- main software-pipelined loop ------------------
        # pv lags one slot behind exp: in slot i, everything (ss(i+1),
        # filler, pv(i-1)) depends only on exp(i-1), which completed at the
        # slot boundary — so the PE queue head never sits on a semaphore and
        # LDWEIGHTS pull-ahead keeps every matmul fill-overlapped.
        emit_ss(0)
        units = []
        for i in range(npos):
            pr, c, kt = positions[i]
            if kt == 0:
                assert not units, f"unemitted units at chunk ({pr},{c})"
                units = qk_units(pr, c)
            # pv first: its streams are long enough to hide the ss pair's
            # LDWEIGHTS behind them (the co-executing ss pair has no room
            # to hide anyone else's weight loads)
            if i >= 2:
                emit_pv(i - 2)
                ppr, pc, pkt = positions[i - 2]
                if pkt == st_n - 1:
                    # must precede this chunk's unit pops: pair-3 units
                    # out-project the chunk this normalize produces
                    normalize(ppr, pc)
            if i + 1 < npos:
                emit_ss(i + 1)
            emit_exp(i)
            if pr == 0 and c == 0:
                proj_v(kt)          # V just-in-time: V[kt] feeds pv slot kt+1
            if units:
                u = units.pop(0)
                if u is not None:
                    u()
        assert not units
        emit_pv(npos - 2)
        emit_pv(npos - 1)
        normalize(MT - 1, qc_n - 1, last=True)
        for et in range(OT):
            outproj_et(qc_n - 1, et)

    nc.compile()
    return nc


def _part_major(a, inner):
    """[K*128, F] -> [128, K, F] with part[p, k, f] = a[k*128+p, f]."""
    k = a.shape[0] // P
    return np.ascontiguousarray(a.reshape(k, P, inner).transpose(1, 0, 2))


def make_in_maps(x, W_qkv, b_qkv, W_out):
    """Host-side sharding/permutation: per-core input dicts."""
    x = np.asarray(x, dtype=np.float32)
    W_qkv = np.asarray(W_qkv, dtype=np.float32)
    b_qkv = np.asarray(b_qkv, dtype=np.float32)
    W_out = np.asarray(W_out, dtype=np.float32)

    # per-head q/k/v rows of the fused projection
    Wh = W_qkv.reshape(H, 3 * HD, D)       # [16, 192, 1024]
    bh = b_qkv.reshape(H, 3 * HD)          # [16, 192]
    Wq_h, Wk_h, Wv_h = Wh[:, 0:HD], Wh[:, HD:2 * HD], Wh[:, 2 * HD:3 * HD]
    bq_h, bk_h, bv_h = bh[:, 0:HD], bh[:, HD:2 * HD], bh[:, 2 * HD:3 * HD]

    in_maps = []
    for core in range(8):
        b = core // 2
        g = core % 2
        hs = slice(8 * g, 8 * g + 8)

        xT = np.ascontiguousarray(x[b].T)                    # [1024, 2048]
        xT_dev = _part_major(xT, S).astype(BF16)             # [128, 8, 2048]

        Wq_core = Wq_h[hs].reshape(DC, D)                    # [512, 1024]
        Wk_core = Wk_h[hs].reshape(DC, D)
        Wv_core = Wv_h[hs].reshape(DC, D)
        wq_dev = _part_major(np.ascontiguousarray(Wq_core.T), DC).astype(BF16)
        wk_dev = _part_major(np.ascontiguousarray(Wk_core.T), DC).astype(BF16)
        wv_dev = _part_major(np.ascontiguousarray(Wv_core.T), DC).astype(BF16)

        Wo_core = W_out[:, DC * g:DC * (g + 1)]              # [1024, 512]
        wo_dev = _part_major(np.ascontiguousarray(Wo_core.T), D).astype(BF16)

        bq_dev = np.ascontiguousarray(
            bq_h[hs].reshape(DC).reshape(MT, P).T).astype(np.float32)
        bk_dev = np.ascontiguousarray(
            bk_h[hs].reshape(DC).reshape(MT, P).T).astype(np.float32)
        bv_dev = np.ascontiguousarray(
            bv_h[hs].reshape(DC).reshape(MT, P).T).astype(np.float32)

        in_maps.append({
            "xT": xT_dev, "wq": wq_dev, "wk": wk_dev, "wv": wv_dev,
            "wo": wo_dev, "bq": bq_dev, "bk": bk_dev, "bv": bv_dev,
        })
    return in_maps


def gather_out(results, b_out):
    """Sum the two per-batch partials, add bias, return [B, S, D] fp32."""
    b_out = np.asarray(b_out, dtype=np.float32)
    out = np.empty((B, S, D), np.float32)
    for b in range(B):
        part = results[2 * b]["out"] + results[2 * b + 1]["out"]   # [128, 8, 2048]
        outT = part.transpose(1, 0, 2).reshape(D, S)               # [1024, 2048]
        out[b] = outT.T + b_out
    return out


_NC_CACHE = {}


def run(x, W_qkv, b_qkv, W_out, b_out, trace=False):
    from concourse import bass_utils

    if "nc" not in _NC_CACHE:
        _NC_CACHE["nc"] = build_nc()
    nc = _NC_CACHE["nc"]

    in_maps = make_in_maps(x, W_qkv, b_qkv, W_out)
    res = bass_utils.run_bass_kernel_spmd(
        nc, in_maps, core_ids=list(range(8)), trace=trace
    )
    out = gather_out(res.results, b_out)
    return out, res


def kernel(x, W_qkv, b_qkv, W_out, b_out):
    out, _ = run(x, W_qkv, b_qkv, W_out, b_out, trace=False)
    return out


# revision 17
# speedup vs baseline: 1.0092x; 1.0092x over previous
# Multi-head attention (B=4, S=2048, D=1024, H=16) on 8 TRN2 NeuronCores.
#
# Sharding: core c handles batch b = c//2 and head-group g = c%2 (8 heads).
# Each core computes Q/K/V projections for its 8 heads, full attention over
# S=2048, and a partial output projection over its 512 value-features.
# Host sums the two partial outputs per batch and adds b_out.
#
# Device math (per core, all matmuls bf16 with fp32 PSUM accumulation):
#   Q^T, K^T  [512, S]   = W_slice @ x^T (+ per-partition bias)
#   V         [S, 512]   = x @ Wv^T  (bias folded into values, see below)
#   S^T tiles [128k, q]  = K_h Q_h^T, two heads row-tiled on the PE array
#   P^T       = exp(S^T * 0.125)                 (ACT engine, bf16 out)
#   [PV^T; l] = [V | 1]^T @ P^T                  (l = softmax denominator)
#   values^T  = PV^T * (1/l) + b_v               (exact: (P(V+b))/l = PV/l + b)
#   out^T     = W_out_slice^T-contraction over 512 features (partial)
#
# v2: software-pipelined emission. The PE queue is strict-FIFO per engine, so
# a semaphore-waiting matmul blocks everything behind it. Emission order per
# kt step is: ss(kt+1) [only waits exp(kt-1)], filler unit, pv(kt) [waits
# exp(kt)]. Projection/out-proj work is split into ~0.4-0.9us units placed
# one per kt so no single filler blob stalls the exp pipeline. reciprocal ->
# reciprocal_approx_fast (the exact one is ~3.3us on one partition and
# clogged the DVE queue, delaying the PSUM-releasing copies behind it).
import sys

for _p in ("/opt/trn_rl_repo",):
    if _p not in sys.path:
        sys.path.insert(0, _p)

import numpy as np
import ml_dtypes

BF16 = ml_dtypes.bfloat16

B, S, D = 4, 2048, 1024
H, HD = 16, 64
P = 128
HC = 8            # heads per core
DC = HC * HD      # 512 value-features per core
KS = D // P       # 8 contraction subtiles for projections
MT = DC // P      # 4 M-tiles == head pairs
ST = S // P       # 16 seq tiles
NQ = 512          # q-chunk (matmul free dim)
QC = S // NQ      # 4
OT = D // P       # 8 output e-tiles
OKS = DC // P     # 4 contraction subtiles for out-proj


def build_nc(s=S):
    """Build the single-core Bass/Tile program (SPMD across 8 cores)."""
    import concourse.tile as tile
    import concourse.mybir as mybir
    from concourse import bacc
    from contextlib import ExitStack

    dt = mybir.dt
    st_n = s // P
    qc_n = s // NQ

    nc = bacc.Bacc("TRN2", target_bir_lowering=False, debug=False)

    xT = nc.dram_tensor("xT", [P, KS, s], dt.bfloat16, kind="ExternalInput").ap()
    wq = nc.dram_tensor("wq", [P, KS, DC], dt.bfloat16, kind="ExternalInput").ap()
    wk = nc.dram_tensor("wk", [P, KS, DC], dt.bfloat16, kind="ExternalInput").ap()
    wv = nc.dram_tensor("wv", [P, KS, DC], dt.bfloat16, kind="ExternalInput").ap()
    wo = nc.dram_tensor("wo", [P, OKS, D], dt.bfloat16, kind="ExternalInput").ap()
    bq = nc.dram_tensor("bq", [P, MT], dt.float32, kind="ExternalInput").ap()
    bk = nc.dram_tensor("bk", [P, MT], dt.float32, kind="ExternalInput").ap()
    bv = nc.dram_tensor("bv", [P, MT], dt.float32, kind="ExternalInput").ap()
    out = nc.dram_tensor("out", [P, OT, s], dt.float32, kind="ExternalOutput").ap()

    with tile.TileContext(nc) as tc, ExitStack() as ctx:
        persist = ctx.enter_context(tc.tile_pool(name="persist", bufs=1))
        # PSUM budget is 8 banks: scores 2x[128,1024] (4) + one shared 4-slot
        # pool of 1-bank tiles for proj/pv/outproj accumulators (4).
        ps_s = ctx.enter_context(tc.tile_pool(name="ps_s", bufs=2, space="PSUM"))
        ps_pv = ctx.enter_context(tc.tile_pool(name="ps_pv", bufs=4, space="PSUM"))
        ps_io = ps_pv
        ptp = ctx.enter_context(tc.tile_pool(name="ptp", bufs=6))
        small = ctx.enter_context(tc.tile_pool(name="small", bufs=6))
        dramp = ctx.enter_context(tc.tile_pool(name="dramp", bufs=4, space="DRAM"))

        xT_sb = persist.tile([P, KS, s], dt.bfloat16, name="xT_sb")
        wq_sb = persist.tile([P, KS, DC], dt.bfloat16, name="wq_sb")
        wk_sb = persist.tile([P, KS, DC], dt.bfloat16, name="wk_sb")
        wv_sb = persist.tile([P, KS, DC], dt.bfloat16, name="wv_sb")
        wo_sb = persist.tile([P, OKS, D], dt.bfloat16, name="wo_sb")
        bq_sb = persist.tile([P, MT], dt.float32, name="bq_sb")
        bk_sb = persist.tile([P, MT], dt.float32, name="bk_sb")
        bv_sb = persist.tile([P, MT], dt.float32, name="bv_sb")
        QT_sb = persist.tile([P, MT, s], dt.bfloat16, name="QT_sb")
        KT_sb = persist.tile([P, MT, s], dt.bfloat16, name="KT_sb")
        V_sb = persist.tile([P, st_n, HC, HD + 1], dt.bfloat16, name="V_sb")
        VL_sb = persist.tile([P, OKS, s], dt.bfloat16, name="VL_sb")

        # Input loads spread across the three DMA-capable queues (sync,
        # scalar, gpsimd) in dependency order, so the first K-projection can
        # start after ~5us instead of waiting out one serialized queue.
        half = s // 2
        for ks in range(KS):
            nc.sync.dma_start(out=wk_sb[:, ks], in_=wk[:, ks])
        nc.sync.dma_start(out=bk_sb, in_=bk)
        nc.sync.dma_start(out=bq_sb, in_=bq)
        for ks in range(KS):
            nc.scalar.dma_start(out=xT_sb[:, ks, 0:half], in_=xT[:, ks, 0:half])
            nc.gpsimd.dma_start(out=xT_sb[:, ks, half:s], in_=xT[:, ks, half:s])
        nc.scalar.dma_start(out=wq_sb, in_=wq)
        nc.gpsimd.dma_start(out=wv_sb, in_=wv)
        nc.gpsimd.dma_start(out=bv_sb, in_=bv)
        nc.scalar.dma_start(out=wo_sb, in_=wo)
        # ones column for the fused softmax-denominator trick
        nc.vector.memset(V_sb[:, :, :, HD:HD + 1], 1.0)

        # ---------- fine-grained filler units (~0.4-0.9us of PE work) ------
        proj_ps = {}

        def proj_qk_half(w_sb, b_sb, dst, mt, c, half, wch):
            # half 0: ks 0-3 (opens the psum group); half 1: ks 4-7 + bias.
            key = (wch, mt, c)
            if half == 0:
                proj_ps[key] = ps_io.tile(
                    [P, NQ], dt.float32, name=f"ps_{wch}_{mt}_{c}", tag="pv"
                )
            ps = proj_ps[key]
            for ks in range(4 * half, 4 * half + 4):
                nc.tensor.matmul(
                    ps,
                    lhsT=w_sb[:, ks, mt * P:(mt + 1) * P],
                    rhs=xT_sb[:, ks, c * NQ:(c + 1) * NQ],
                    start=(ks == 0),
                    stop=(ks == KS - 1),
                    skip_group_check=True,
                )
            if half == 1:
                nc.vector.tensor_add(
                    dst[:, mt, c * NQ:(c + 1) * NQ],
                    ps,
                    b_sb[:, mt:mt + 1].to_broadcast((P, NQ)),
                )
                del proj_ps[key]

        def proj_v(st):
            # V[s-tile, :] = x @ Wv^T (no bias here; folded into values)
            ps = ps_io.tile([P, DC], dt.float32, name=f"ps_v_{st}", tag="pv")
            for ks in range(KS):
                nc.tensor.matmul(
                    ps,
                    lhsT=xT_sb[:, ks, st * P:(st + 1) * P],
                    rhs=wv_sb[:, ks, :],
                    start=(ks == 0),
                    stop=(ks == KS - 1),
                )
            nc.vector.tensor_copy(
                out=V_sb[:, st, :, 0:HD],
                in_=ps.rearrange("p (h d) -> p h d", h=HC),
            )

        def outproj_et(c, et):
            # partial out-projection over this core's 512 value-features
            po = ps_io.tile([P, NQ], dt.float32, name=f"po_{et}_{c}", tag="pv")
            for ks in range(OKS):
                nc.tensor.matmul(
                    po,
                    lhsT=wo_sb[:, ks, et * P:(et + 1) * P],
                    rhs=VL_sb[:, ks, c * NQ:(c + 1) * NQ],
                    start=(ks == 0),
                    stop=(ks == OKS - 1),
                )
            ot_sb = small.tile([P, NQ], dt.float32, name=f"ot_{et}_{c}", tag="ot")
            nc.vector.tensor_copy(out=ot_sb, in_=po)
            eng = nc.sync if et % 2 == 0 else nc.gpsimd
            eng.dma_start(out=out[:, et, c * NQ:(c + 1) * NQ], in_=ot_sb)

        # ------------------- attention pipeline pieces ---------------------
        positions = [(pr, c, kt)
                     for pr in range(MT) for c in range(qc_n)
                     for kt in range(st_n)]
        npos = len(positions)
        ss_tiles = {}
        pt_tiles = {}
        pv_tiles = {}

        def emit_ss(i):
            pr, c, kt = positions[i]
            cs = slice(c * NQ, (c + 1) * NQ)
            ks_sl = slice(kt * P, (kt + 1) * P)
            ss = ps_s.tile([P, 2 * NQ], dt.float32, name=f"ss_{i}", tag="ss")
            ss_tiles[i] = ss
            # S^T = K_h Q_h^T for the two heads, row-tiled (K=64 each)
            nc.tensor.matmul(
                ss[:, 0:NQ],
                lhsT=KT_sb[0:HD, pr, ks_sl],
                rhs=QT_sb[0:HD, pr, cs],
            )
            nc.tensor.matmul(
                ss[:, NQ:2 * NQ],
                lhsT=KT_sb[HD:P, pr, ks_sl],
                rhs=QT_sb[HD:P, pr, cs],
            )

        def emit_exp(i):
            pt = ptp.tile([P, 2 * NQ], dt.bfloat16, name=f"pt_{i}", tag="pt")
            pt_tiles[i] = pt
            nc.scalar.activation(
                pt, ss_tiles.pop(i), mybir.ActivationFunctionType.Exp, scale=0.125
            )

        def emit_pv(i):
            pr, c, kt = positions[i]
            if kt == 0:
                pv_tiles[(pr, c)] = (
                    ps_pv.tile([HD + 1, NQ], dt.float32, name=f"pvA_{pr}_{c}", tag="pv"),
                    ps_pv.tile([HD + 1, NQ], dt.float32, name=f"pvB_{pr}_{c}", tag="pv"),
                )
            pvA, pvB = pv_tiles[(pr, c)]
            pt = pt_tiles.pop(i)
            nc.tensor.matmul(
                pvA,
                lhsT=V_sb[:, kt, 2 * pr, :],
                rhs=pt[:, 0:NQ],
                start=(kt == 0),
                stop=(kt == st_n - 1),
            )
            nc.tensor.matmul(
                pvB,
                lhsT=V_sb[:, kt, 2 * pr + 1, :],
                rhs=pt[:, NQ:2 * NQ],
                start=(kt == 0),
                stop=(kt == st_n - 1),
            )

        def normalize(pr, c, last=False):
            # values^T = PV^T * (1/l) + b_v; head B is DMA-shifted to
            # partitions 64..127 so out-proj sees [128, s] rhs tiles.
            # Both PSUM banks are evacuated FIRST (frees them for the next
            # chunk's accumulators), then the two slow reciprocals, then the
            # DMA-broadcast bounce; the SBUF-only muls/adds run on the idle
            # GPSIMD engine so they can't clog the DVE queue behind them.
            cs = slice(c * NQ, (c + 1) * NQ)
            pvA, pvB = pv_tiles.pop((pr, c))
            pvsA = small.tile([HD + 1, NQ], dt.float32, name=f"pvs_{pr}_{c}_0", tag="pvs")
            pvsB = small.tile([HD + 1, NQ], dt.float32, name=f"pvs_{pr}_{c}_1", tag="pvs")
            nc.vector.tensor_copy(out=pvsA, in_=pvA)
            nc.vector.tensor_copy(out=pvsB, in_=pvB)
            # Both l rows live on partition 64 of their psum tiles; DMA-gather
            # them onto two distinct partitions so ONE reciprocal handles both
            # (the iterative divide costs ~3.4us per 512 elems per lane no
            # matter how many lanes run, so batching halves the DVE cost).
            lr = small.tile([2, NQ], dt.float32, name=f"lr_{pr}_{c}", tag="lr")
            nc.sync.dma_start(out=lr[0:1, :], in_=pvsA[HD:HD + 1, :])
            nc.sync.dma_start(out=lr[1:2, :], in_=pvsB[HD:HD + 1, :])
            rec = small.tile([2, NQ], dt.float32, name=f"r_{pr}_{c}", tag="rec")
            nc.vector.reciprocal(rec, lr)
            ld = dramp.tile([2, NQ], dt.float32, name=f"ld_{pr}_{c}", tag="ld")
            nc.sync.dma_start(out=ld, in_=rec)
            for half, pvs in ((0, pvsA), (1, pvsB)):
                # DMA-broadcast 1/l down to partitions 0..63 (SBUF APs can't
                # have stride-0 partition dims, DRAM APs can: bounce via DRAM).
                bc = small.tile([HD, NQ], dt.float32,
                                name=f"bc_{pr}_{c}_{half}", tag="bc")
                nc.sync.dma_start(out=bc, in_=ld[half:half + 1, :].to_broadcast((HD, NQ)))
                ee = nc.vector if last else nc.gpsimd
                if half == 0:
                    ee.tensor_mul(VL_sb[0:HD, pr, cs], pvs[0:HD, :], bc)
                else:
                    stg = small.tile([HD, NQ], dt.bfloat16,
                                     name=f"st_{pr}_{c}", tag="stg")
                    ee.tensor_mul(stg, pvs[0:HD, :], bc)
                    nc.sync.dma_start(out=VL_sb[HD:P, pr, cs], in_=stg)
            (nc.vector if last else nc.gpsimd).tensor_add(
                VL_sb[:, pr, cs],
                VL_sb[:, pr, cs],
                bv_sb[:, pr:pr + 1].to_broadcast((P, NQ)),
            )

        # ------------------- filler worklists ------------------------------
        # One unit popped per kt. Deadlines honored by construction:
        #  - K(pr+1), Q(pr+1, c0) finish during pair pr
        #  - Q(pr, c+1) finishes during chunk (pr, c)
        #  - V is emitted just-in-time inside pair 0 chunk 0 (not a unit)
        #  - out-proj of query-chunk c-1 runs through pair 3's chunk c
        def qk_units(pr, c):
            us = []
            if c + 1 < qc_n:  # this pair's next q-chunk
                us += [lambda h=h, cc=c + 1: proj_qk_half(
                    wq_sb, bq_sb, QT_sb, pr, cc, h, "q") for h in range(2)]
            if pr + 1 < MT:
                if c < 2:     # next pair's K, spread over chunks 0-1
                    for kc in (2 * c, 2 * c + 1):
                        us += [lambda h=h, kc=kc: proj_qk_half(
                            wk_sb, bk_sb, KT_sb, pr + 1, kc, h, "k")
                            for h in range(2)]
                elif c == 2:  # next pair's first q-chunk
                    us += [lambda h=h: proj_qk_half(
                        wq_sb, bq_sb, QT_sb, pr + 1, 0, h, "q")
                        for h in range(2)]
            if pr == MT - 1 and c >= 1:
                # defer to the chunk's second half: the previous chunk's
                # normalize chain (evac -> recip -> bounce -> mul) needs ~8us
                # before VL is readable; an early out-proj would sem-block
                # the PE queue on it
                us += [None] * (8 - len(us))
                us += [lambda et=et, cc=c - 1: outproj_et(cc, et)
                       for et in range(OT)]
            return us

        # ------------------- preamble --------------------------------------
        for c in range(qc_n):
            for h in range(2):
                proj_qk_half(wk_sb, bk_sb, KT_sb, 0, c, h, "k")
        for h in range(2):
            proj_qk_half(wq_sb, bq_sb, QT_sb, 0, 0, h, "q")

        # ------------------- main software-pipelined loop ------------------
        # pv lags one slot behind exp: in slot i, everything (ss(i+1),
        # filler, pv(i-1)) depends only on exp(i-1), which completed at the
        # slot boundary — so the PE queue head never sits on a semaphore and
        # LDWEIGHTS pull-ahead keeps every matmul fill-overlapped.
        emit_ss(0)
        units = []
        for i in range(npos):
            pr, c, kt = positions[i]
            if kt == 0:
                assert not units, f"unemitted units at chunk ({pr},{c})"
                units = qk_units(pr, c)
            if i + 1 < npos:
                emit_ss(i + 1)
            emit_exp(i)
            if pr == 0 and c == 0:
                proj_v(kt)          # V just-in-time: V[kt] feeds pv slot kt+1
            if i >= 2:
                emit_pv(i - 2)
                ppr, pc, pkt = positions[i - 2]
                if pkt == st_n - 1:
                    # must precede this chunk's unit pops: pair-3 units
                    # out-project the chunk this normalize produces
                    normalize(ppr, pc)
            if units:
                u = units.pop(0)
                if u is not None:
                    u()
        assert not units
        emit_pv(npos - 2)
        emit_pv(npos - 1)
        normalize(MT - 1, qc_n - 1, last=True)
        for et in range(OT):
            outproj_et(qc_n - 1, et)

    nc.compile()
    return nc


def _part_major(a, inner):
    """[K*128, F] -> [128, K, F] with part[p, k, f] = a[k*128+p, f]."""
    k = a.shape[0] // P
    return np.ascontiguousarray(a.reshape(k, P, inner).transpose(1, 0, 2))


def make_in_maps(x, W_qkv, b_qkv, W_out):
    """Host-side sharding/permutation: per-core input dicts."""
    x = np.asarray(x, dtype=np.float32)
    W_qkv = np.asarray(W_qkv, dtype=np.float32)
    b_qkv = np.asarray(b_qkv, dtype=np.float32)
    W_out = np.asarray(W_out, dtype=np.float32)

    # per-head q/k/v rows of the fused projection
    Wh = W_qkv.reshape(H, 3 * HD, D)       # [16, 192, 1024]
    bh = b_qkv.reshape(H, 3 * HD)          # [16, 192]
    Wq_h, Wk_h, Wv_h = Wh[:, 0:HD], Wh[:, HD:2 * HD], Wh[:, 2 * HD:3 * HD]
    bq_h, bk_h, bv_h = bh[:, 0:HD], bh[:, HD:2 * HD], bh[:, 2 * HD:3 * HD]

    in_maps = []
    for core in range(8):
        b = core // 2
        g = core % 2
        hs = slice(8 * g, 8 * g + 8)

        xT = np.ascontiguousarray(x[b].T)                    # [1024, 2048]
        xT_dev = _part_major(xT, S).astype(BF16)             # [128, 8, 2048]

        Wq_core = Wq_h[hs].reshape(DC, D)                    # [512, 1024]
        Wk_core = Wk_h[hs].reshape(DC, D)
        Wv_core = Wv_h[hs].reshape(DC, D)
        wq_dev = _part_major(np.ascontiguousarray(Wq_core.T), DC).astype(BF16)
        wk_dev = _part_major(np.ascontiguousarray(Wk_core.T), DC).astype(BF16)
        wv_dev = _part_major(np.ascontiguousarray(Wv_core.T), DC).astype(BF16)

        Wo_core = W_out[:, DC * g:DC * (g + 1)]              # [1024, 512]
        wo_dev = _part_major(np.ascontiguousarray(Wo_core.T), D).astype(BF16)

        bq_dev = np.ascontiguousarray(
            bq_h[hs].reshape(DC).reshape(MT, P).T).astype(np.float32)
        bk_dev = np.ascontiguousarray(
            bk_h[hs].reshape(DC).reshape(MT, P).T).astype(np.float32)
        bv_dev = np.ascontiguousarray(
            bv_h[hs].reshape(DC).reshape(MT, P).T).astype(np.float32)

        in_maps.append({
            "xT": xT_dev, "wq": wq_dev, "wk": wk_dev, "wv": wv_dev,
            "wo": wo_dev, "bq": bq_dev, "bk": bk_dev, "bv": bv_dev,
        })
    return in_maps


def gather_out(results, b_out):
    """Sum the two per-batch partials, add bias, return [B, S, D] fp32."""
    b_out = np.asarray(b_out, dtype=np.float32)
    out = np.empty((B, S, D), np.float32)
    for b in range(B):
        part = results[2 * b]["out"] + results[2 * b + 1]["out"]   # [128, 8, 2048]
        outT = part.transpose(1, 0, 2).reshape(D, S)               # [1024, 2048]
        out[b] = outT.T + b_out
    return out


_NC_CACHE = {}


def run(x, W_qkv, b_qkv, W_out, b_out, trace=False):
    from concourse import bass_utils

    if "nc" not in _NC_CACHE:
        _NC_CACHE["nc"] = build_nc()
    nc = _NC_CACHE["nc"]

    in_maps = make_in_maps(x, W_qkv, b_qkv, W_out)
    res = bass_utils.run_bass_kernel_spmd(
        nc, in_maps, core_ids=list(range(8)), trace=trace
    )
    out = gather_out(res.results, b_out)
    return out, res


def kernel(x, W_qkv, b_qkv, W_out, b_out):
    out, _ = run(x, W_qkv, b_qkv, W_out, b_out, trace=False)
    return out
